# revision 27
# baseline (speedup 1.0000x reference)
"""mLSTM block kernel for Trainium2, 8 NeuronCores.

Sharding: batch (4) x head-halves (2) -> 8 cores. Each core handles one
batch element and 8 of the 16 heads: RMSNorm + qkv/gate projections +
chunked mLSTM recurrence (chunk=128) + its half of the output projection.
Host adds the two per-batch partial output projections + residual.

Math (per head, chunk of C=128 positions, inclusive cumsums, all
exponent arguments <= 0 so exp never overflows):
  nlogf = softplus(-softcap(gf)),  NL_t = cumsum(nlogf)
  nlogi, nlogo likewise;  Z_t = NL_t + nlogo_t   (folds o_t = exp(-nlogo))
  r_s = NL_s - nlogi_s
  A'[s,t] = (k_s . q_t) * exp(r_s - Z_t)  for t >= s else 0
  h_t = sum_s A'[s,t] v_s + (q_t * exp(-Z_t)) . S_chunk_start
  S <- exp(-NL_C) * S + sum_s exp(NL_s - NL_C - nlogi_s) k_s v_s^T

Decay matrices: dm = min(r_s + (-Z_t), 0) via per-head Pool tensor_scalar
(the min guards exp overflow in the invalid s>t half), Exp on ACT, then a
causal 0/1 mask multiply in fp16 (DVE 4x mode). PSUM evacuations for qkT
and the output projection run on the otherwise-idle Pool engine.

RMSNorm trick: xn = inv[s] * x[:, s], so projections run on raw bf16 x
and are scaled by inv at PSUM-evacuation; w_rms is folded into weights
on the host. Gate projections use bf16 hi/lo splitting (3 matmul
groups) because decay errors random-walk over the 128-step cumsum.
"""

import sys

sys.path.insert(0, "/opt/trn_rl_repo")

import numpy as np
import ml_dtypes

import concourse.bass as bass
import concourse.mybir as mybir
import concourse.tile as tile
from concourse import bass_utils
import bass_rust

F32 = mybir.dt.float32
BF16 = mybir.dt.bfloat16
FP16 = mybir.dt.float16
AF = mybir.ActivationFunctionType
OP = mybir.AluOpType

B, S, D = 4, 2048, 1024
H, DH = 16, 64
HL = 8              # heads per core
C = 128             # chunk length
NCHUNK = S // C     # 16
KO = D // 128       # 8 k-tiles over D
FL = HL * DH        # 512 local feature width
EPS = 1e-6
CAP = 15.0
NEG = -30000.0

# ---------------------------------------------------------------------------
# walrus workaround: this compiler build only accepts 1 sync-wait on
# CTRL-lowered instructions (Drain/EventSemaphore/Nop) and rejects >2
# elsewhere. Move the excess onto EventSemaphore carriers inserted before.
# ---------------------------------------------------------------------------
_CTRL_INSTS = ("InstDrain", "InstEventSemaphore", "InstNop")


def _split_excess_waits(nc, max_waits=1):
    n_new = 0
    for fn in nc.m.functions:
        for bb in fn.blocks:
            lst = bb.instructions
            i = 0
            while i < len(lst):
                inst = lst[i]
                si = inst.sync_info
                cap = 1 if type(inst).__name__ in _CTRL_INSTS else max_waits
                if si is None or len(si.on_wait) <= cap:
                    i += 1
                    continue
                waits = list(si.on_wait)
                keep, excess = waits[:cap], waits[cap:]
                carriers = []
                for j in range(len(excess)):
                    ev = mybir.InstEventSemaphore(
                        name=f"I-waitfix-{n_new}", ins=[], outs=[])
                    n_new += 1
                    ev.engine = inst.engine
                    ev.sync_info = bass_rust.SyncInfo(
                        on_wait=[excess[j]], on_update=[])
                    nc.register_instruction(ev, overwrite=True)
                    carriers.append(ev)
                si.on_wait = keep
                for k, ev in enumerate(carriers):
                    lst.insert(i + k, ev)
                i += len(carriers) + 1
    return n_new


# ---------------------------------------------------------------------------
# kernel builder
# ---------------------------------------------------------------------------

def build_nc():
    nc = bass.Bass(target_bir_lowering=False, trn_type="TRN2")

    # wqkg = [wqk(1024) | wgh(24)] along columns; wv separate (freed early)
    WALL = 2 * FL + 3 * HL
    xhi_d = nc.dram_tensor("xhi", [D, S], FP16, kind="ExternalInput")
    wall_d = nc.dram_tensor("wall", [D, WALL], FP16, kind="ExternalInput")
    wv_d = nc.dram_tensor("wv", [D, FL], FP16, kind="ExternalInput")
    wo_d = nc.dram_tensor("wo", [FL, D], FP16, kind="ExternalInput")
    s0_d = nc.dram_tensor("s0", [HL, DH, DH], FP16, kind="ExternalInput")
    # cf32 = [triu | e127 | maskneg | idf32], cf16 = [idbf | onesb | mask01]
    cf32_d = nc.dram_tensor("cf32", [128, 4, 128], F32, kind="ExternalInput")
    cf16_d = nc.dram_tensor("cf16", [128, 3, 128], FP16, kind="ExternalInput")
    out_d = nc.dram_tensor("outT", [D, S], FP16, kind="ExternalOutput")

    xhi_r = xhi_d.rearrange("(ko p) s -> p ko s", p=128)

    with tile.TileContext(nc) as tc:
        with (
            tc.tile_pool(name="wk", bufs=3) as wk,
            tc.tile_pool(name="dram", bufs=1, space="DRAM") as dpool,
            tc.tile_pool(name="ps_big", bufs=2, space="PSUM") as ps_big,
            tc.tile_pool(name="ps_kt", bufs=2, space="PSUM") as ps_kt,
            tc.tile_pool(name="ps_st", bufs=2, space="PSUM") as ps_st,
        ):
            # ---- persistent tiles --------------------------------------
            frees = []

            def ptile(shape, dtype, name):
                t, f = tc.tile(shape, dtype, name=name)
                frees.append(f)
                return t

            cf32 = ptile([128, 4, 128], F32, "cf32")
            triu, e127, maskneg, idf = (cf32[:, i, :] for i in range(4))
            cf16 = ptile([128, 3, 128], FP16, "cf16")
            idb, onesb, mask01 = (cf16[:, i, :] for i in range(3))
            wall = ptile([128, KO, WALL], FP16, "wall")
            wqk = wall[:, :, 0:2 * FL]
            wgh = wall[:, :, 2 * FL:2 * FL + 3 * HL]
            wo = ptile([128, FL // 128, D], FP16, "wo")
            s_all = ptile([128, 2, HL // 2, DH], FP16, "s_all")
            li1_cols = ptile([128, NCHUNK], F32, "li1_cols")
            li2_cols = ptile([128, NCHUNK], F32, "li2_cols")
            gsp = ptile([128, NCHUNK, 24], F32, "gsp")
            nl_cols = ptile([128, 128], F32, "nl_cols")
            r_cols = ptile([128, 128], F32, "r_cols")
            z_cols = ptile([128, 128], F32, "z_cols")
            w_cols = ptile([128, 128], F32, "w_cols")
            exp_nlc = ptile([128, 128], F32, "exp_nlc")
            qkT = ptile([128, KO, S], FP16, "qkT")
            vpos = ptile([128, NCHUNK, FL], FP16, "vpos")
            eps_col = ptile([128, 1], F32, "eps_col")
            nc.vector.memset(eps_col[:], EPS)
            xhi, xhi_free = tc.tile([128, KO, S], FP16, name="xhi")
            wv, wv_free = tc.tile([128, KO, FL], FP16, name="wv")
            # freed before hT allocation (only needed for decay tables)
            lnv_rep, lnv_free = tc.tile([128, S], F32, name="lnv_rep")
            g_cols, g_free = tc.tile([128, NCHUNK, 24], F32, name="g_cols")

            # ---- loads (ordered so PE can start earliest) --------------
            nc.sync.dma_start(cf32[:], cf32_d[:])
            nc.sync.dma_start(cf16[:], cf16_d[:])
            # first s-block of x unblocks variance-sb0 at ~3us
            nc.sync.dma_start(xhi[:, :, 0:512], xhi_r[:, :, 0:512])
            nc.sync.dma_start(wall[:], wall_d.rearrange("(ko p) j -> p ko j", p=128))
            for sb in range(1, 4):
                nc.sync.dma_start(xhi[:, :, sb * 512:(sb + 1) * 512],
                                  xhi_r[:, :, sb * 512:(sb + 1) * 512])
            nc.sync.dma_start(wv[:], wv_d.rearrange("(ko p) j -> p ko j", p=128))
            nc.sync.dma_start(wo[:], wo_d.rearrange("(ko p) j -> p ko j", p=128))
            # state init: head h -> partitions (h%2)*64+d, index h//2
            nc.sync.dma_start(
                s_all[:, 0, :, :],
                s0_d.rearrange("(a b) d e -> (b d) a e", b=2))

            # ---- projection group emitters -----------------------------
            def emit_qk_group(jt, sb):
                pq = ps_big.tile([128, 512], F32, name="pq", tag="big")
                for k in range(KO):
                    nc.tensor.matmul(
                        pq[:], wqk[:, k, jt * 128:(jt + 1) * 128],
                        xhi[:, k, sb * 512:(sb + 1) * 512],
                        start=(k == 0), stop=(k == KO - 1))
                nc.scalar.copy(
                    qkT[:, jt, sb * 512:(sb + 1) * 512], pq[:])

            def emit_vpos_group(st):
                pvp = ps_big.tile([128, 512], F32, name="pvp", tag="big")
                for k in range(KO):
                    nc.tensor.matmul(
                        pvp[:], xhi[:, k, st * 128:(st + 1) * 128],
                        wv[:, k, :],
                        start=(k == 0), stop=(k == KO - 1))
                nc.scalar.copy(vpos[:, st, :], pvp[:])

            # ---- variance + inv std (sb0 first: only needs xhi-sb0) ----
            def emit_var(sb):
                pv = ps_kt.tile([128, 512], F32, name="pv", tag="kt")
                for k in range(KO):
                    xsq = wk.tile([128, 512], FP16, name="xsq", tag="xsq", bufs=2)
                    xs = xhi[:, k, sb * 512:(sb + 1) * 512]
                    nc.vector.tensor_tensor(xsq[:], xs, xs, OP.mult)
                    nc.tensor.matmul(pv[:], onesb, xsq[:],
                                     start=(k == 0), stop=(k == KO - 1))
                nc.scalar.activation(lnv_rep[:, sb * 512:(sb + 1) * 512],
                                     pv[:], AF.Ln, bias=eps_col[:],
                                     scale=1.0 / D)

            emit_var(0)

            # q|k projection for the first s-block, ahead of everything
            for jt in range(KO):
                emit_qk_group(jt, 0)

            for sb in range(1, 4):
                emit_var(sb)

            # ln(var)/D columns for decay-table folding: [128 s, st]
            for st in range(NCHUNK):
                pt = ps_kt.tile([128, 128], F32, name="pt", tag="kt")
                nc.tensor.transpose(
                    pt[:, 0:1], lnv_rep[0:1, st * 128:(st + 1) * 128],
                    cf32[0:1, 3, 0:1])
                # ln(inv) = -0.5*lnv ; store -2*ln(inv)=lnv and -ln(inv)
                nc.scalar.activation(li1_cols[:, st:st + 1], pt[:, 0:1],
                                     AF.Copy, scale=-1.0)
                nc.scalar.activation(li2_cols[:, st:st + 1], pt[:, 0:1],
                                     AF.Copy, scale=-0.5)

            # ---- gate projections (row-major, hi/lo compensated) -------
            for sb in range(4):
                pg = ps_kt.tile([24, 512], F32, name="pg", tag="kt")
                for k in range(KO):
                    nc.tensor.matmul(pg[:], wgh[:, k, :],
                                     xhi[:, k, sb * 512:(sb + 1) * 512],
                                     start=(k == 0), stop=(k == KO - 1))
                inv24 = wk.tile([24, 512], F32, name="inv24", tag="inv24",
                                bufs=2)
                nc.scalar.activation(inv24[:],
                                     lnv_rep[0:24, sb * 512:(sb + 1) * 512],
                                     AF.Exp, scale=-0.5)
                graw = wk.tile([24, 512], F32, name="graw", tag="graw",
                               bufs=2)
                nc.vector.tensor_tensor(graw[:], pg[:], inv24[:], OP.mult)
                for j in range(4):
                    st = sb * 4 + j
                    pt = ps_kt.tile([128, 128], F32, name="pt", tag="kt")
                    nc.tensor.transpose(
                        pt[:, 0:24], graw[:, j * 128:(j + 1) * 128],
                        cf32[0:24, 3, 0:24])
                    nc.scalar.copy(g_cols[:, st, :], pt[:, 0:24])
            # softcap -> log-gates: gsp = ln(sigmoid(15*tanh(g/15))) <= 0
            # (CoreSim lacks Softplus; -gsp is the neg-log gate)
            t1 = wk.tile([128, NCHUNK, 24], F32, name="t1", tag="t1",
                         bufs=1)
            nc.scalar.activation(t1[:], g_cols[:], AF.Tanh, scale=1.0 / CAP)
            sg = wk.tile([128, NCHUNK, 24], F32, name="sg", tag="sg",
                         bufs=1)
            nc.scalar.activation(sg[:], t1[:], AF.Sigmoid, scale=CAP)
            nc.scalar.activation(gsp[:], sg[:], AF.Ln)
            lns_i = gsp[:, :, 0:HL]
            lns_f = gsp[:, :, HL:2 * HL]
            lns_o = gsp[:, :, 2 * HL:3 * HL]

            # ---- cumsums / decay tables --------------------------------
            # triu holds -1 on s<=t, so NL = -cumsum(ln f) >= 0
            pnl = ps_kt.tile([128, 512], F32, name="pnl", tag="kt")
            nc.tensor.matmul(pnl[:, 0:128], triu, lns_f,
                             start=True, stop=True)
            nc.vector.tensor_copy(nl_cols[:], pnl[:, 0:128])
            nc.vector.tensor_tensor(r_cols[:], nl_cols[:], lns_i, OP.add)
            nc.vector.tensor_tensor(
                r_cols[:], r_cols[:],
                li1_cols[:, :, None].to_broadcast((128, NCHUNK, HL)),
                OP.add)
            # z_cols holds the NEGATED exponent -Z = lns_o - NL + ln(inv)
            nc.vector.tensor_tensor(z_cols[:], lns_o, nl_cols[:], OP.subtract)
            nc.vector.tensor_tensor(
                z_cols[:], z_cols[:],
                li2_cols[:, :, None].to_broadcast((128, NCHUNK, HL)),
                OP.add)
            pnlc = ps_kt.tile([128, 512], F32, name="pnlc", tag="kt")
            nc.tensor.matmul(pnlc[:, 0:128], e127, nl_cols[:],
                             start=True, stop=True)
            w_tmp = wk.tile([128, 128], F32, name="w_tmp", tag="tmp")
            nc.vector.tensor_tensor(w_tmp[:], r_cols[:], pnlc[:, 0:128],
                                    OP.subtract)
            nc.scalar.activation(w_cols[:], w_tmp[:], AF.Exp)
            nc.scalar.activation(exp_nlc[:], pnlc[:, 0:128], AF.Exp,
                                 scale=-1.0)
            # Z rows -> DRAM for partition-broadcast loads
            pzr = ps_kt.tile([128, 128], F32, name="pzr", tag="kt")
            nc.tensor.transpose(pzr[:], z_cols[:], idf)
            zr_sb = wk.tile([128, 128], F32, name="zr_sb", tag="tmp")
            nc.vector.tensor_copy(zr_sb[:], pzr[:])
            zr_dram = dpool.tile([128, 128], F32, name="zr_dram")
            nc.sync.dma_start(zr_dram[:], zr_sb[:])

            # ---- v projection: two chunks ahead; rest dripped in-loop --
            for st in range(2):
                emit_vpos_group(st)

            g_free()
            lnv_free()
            hT, hT_free = tc.tile([128, FL // 128, S], FP16, name="hT")

            # ---- recurrence over chunks --------------------------------
            # waves of 4 heads sharing a partition window: h = 2k + h0
            w_v = w_cols.rearrange("s (c k two) -> s c k two", k=4, two=2)
            e_v = exp_nlc.rearrange("s (c k two) -> s c k two", k=4, two=2)
            for c in range(NCHUNK):
                if c % 2 == 0:
                    repz = wk.tile([128, 2, 4, 2, 128], F32, name="repz",
                                   tag="repz", bufs=2)
                    nc.sync.dma_start(
                        repz[:],
                        zr_dram[None, c * HL:(c + 2) * HL, :]
                        .to_broadcast((128, 2 * HL, 128))
                        .rearrange("s (a k two) t -> s a k two t", k=4, two=2))
                    repez = wk.tile([128, 2, 4, 2, 128], FP16, name="repez",
                                    tag="repez", bufs=2)
                    nc.scalar.activation(repez[:], repz[:], AF.Exp)
                cs = slice(c * 128, (c + 1) * 128)
                # decay matrices: dm = min(r_s + (-Z_t), 0) per head on Pool
                # (min guards exp overflow on the invalid s>t half), then
                # exp on ACT and causal 0/1 mask in fp16 (DVE 4x).
                dm = wk.tile([128, 4, 2, 128], F32, name="dm", tag="d1",
                             bufs=2)
                for k in range(4):
                    for two in range(2):
                        col = c * 8 + k * 2 + two
                        nc.gpsimd.tensor_scalar(
                            dm[:, k, two, :], repz[:, c % 2, k, two, :],
                            r_cols[:, col:col + 1], 0.0, OP.add, OP.min)
                expd = wk.tile([128, 4, 2, 128], FP16, name="expd",
                               tag="expd", bufs=3)
                nc.scalar.activation(expd[:], dm[:], AF.Exp)
                nc.vector.tensor_tensor(
                    expd[:], expd[:],
                    mask01[:, None, None, :].to_broadcast((128, 4, 2, 128)),
                    OP.mult)
                for h0 in range(2):
                    po = h0 * 64
                    pw = slice(po, po + 64)
                    qs_w = qkT[pw, 0:4, cs]
                    ks_w = qkT[pw, 4:8, cs]
                    rez_w = repez[pw, c % 2, :, h0, :]
                    wc_bc = w_v[:, c, :, h0, None].to_broadcast((128, 4, 64))
                    en_bc = e_v[pw, c, :, h0, None].to_broadcast((64, 4, 64))
                    s_old_w = s_all[pw, c % 2, :, :]
                    s_new_w = s_all[pw, (c + 1) % 2, :, :]

                    # attention scores (k.q), 4 heads into one bank
                    pa = ps_big.tile([128, 4, 128], F32, name="pa", tag="pa")
                    for k in range(4):
                        nc.tensor.matmul(pa[:, k, :], ks_w[:, k, :],
                                         qs_w[:, k, :], start=True, stop=True,
                                         skip_group_check=True)
                    pkp = ps_st.tile([128, 4, DH], FP16, name="pkp",
                                     tag="st2")
                    for k in range(4):
                        nc.tensor.matmul(pkp[:, k, :], ks_w[:, k, :],
                                         idb[pw, pw], is_transpose=True,
                                         skip_group_check=True)
                    a_w = wk.tile([128, 4, 128], FP16, name="a_w", tag="a_w",
                                  bufs=2)
                    nc.vector.tensor_tensor(a_w[:], pa[:],
                                            expd[:, :, h0, :], OP.mult)
                    qt_w = wk.tile([128, 4, 128], FP16, name="qt_w",
                                   tag="qt_w", bufs=2)
                    nc.vector.tensor_tensor(qt_w[pw, :, :], qs_w, rez_w,
                                            OP.mult)

                    # hT[e,t] = V^T A' + S^T (q*exp(-Z))
                    pht = ps_kt.tile([128, 4, 128], F32, name="pht",
                                     tag="kt")
                    for k in range(4):
                        vs = vpos[:, c, (2 * k + h0) * DH:
                                  (2 * k + h0 + 1) * DH]
                        nc.tensor.matmul(pht[pw, k, :], vs, a_w[:, k, :],
                                         start=True, stop=False,
                                         tile_position=(0, po),
                                         skip_group_check=True)
                        nc.tensor.matmul(pht[pw, k, :], s_old_w[:, k, :],
                                         qt_w[pw, k, :],
                                         start=False, stop=True,
                                         tile_position=(po, po),
                                         skip_group_check=True)
                    nc.scalar.copy(hT[pw, :, cs], pht[pw, :, :])

                    # state update: S <- exp(-NL_C) S + K~^T V
                    ktb = wk.tile([128, 4, DH], FP16, name="ktb", tag="ktb")
                    nc.vector.tensor_tensor(ktb[:], pkp[:], wc_bc, OP.mult)
                    pst = ps_st.tile([128, 4, DH], F32, name="pst",
                                     tag="st2")
                    for k in range(4):
                        vs = vpos[:, c, (2 * k + h0) * DH:
                                  (2 * k + h0 + 1) * DH]
                        nc.tensor.matmul(pst[pw, k, :], ktb[:, k, :], vs,
                                         start=True, stop=True,
                                         tile_position=(0, po),
                                         skip_group_check=True)
                    sdec = wk.tile([128, 4, DH], F32, name="sdec", tag="sdec")
                    nc.gpsimd.tensor_tensor(sdec[pw, :, :], s_old_w, en_bc,
                                            OP.mult)
                    nc.vector.tensor_tensor(s_new_w, sdec[pw, :, :],
                                            pst[pw, :, :], OP.add)

                # drip-feed v projection two chunks ahead
                if c < NCHUNK - 2:
                    emit_vpos_group(c + 2)

                # drip-feed remaining q|k projection groups (sb = 1..4)
                if c < 12:
                    sb_n = 1 + c // 4
                    emit_qk_group(2 * (c % 4), sb_n)
                    emit_qk_group(2 * (c % 4) + 1, sb_n)

                # interleave output projection once its s-block is complete
                if c % 4 == 3:
                    sb = c // 4
                    osb = wk.tile([128, KO, 512], FP16, name="osb",
                                  tag="osb", bufs=1)
                    for jt in range(KO):
                        pout = ps_big.tile([128, 512], F32, name="pout",
                                           tag="big")
                        for kk in range(FL // 128):
                            nc.tensor.matmul(
                                pout[:], wo[:, kk, jt * 128:(jt + 1) * 128],
                                hT[:, kk, sb * 512:(sb + 1) * 512],
                                start=(kk == 0), stop=(kk == FL // 128 - 1))
                        nc.vector.tensor_copy(osb[:, jt, :], pout[:])
                    nc.sync.dma_start(
                        out_d.rearrange("(jt p) s -> p jt s", p=128)
                        [:, :, sb * 512:(sb + 1) * 512], osb[:])

            hT_free()
            wv_free()
            xhi_free()
            for f in reversed(frees):
                f()

    _split_excess_waits(nc)
    nc.finalize()
    return nc


# ---------------------------------------------------------------------------
# host-side constants and shard prep
# ---------------------------------------------------------------------------

def _consts():
    i = np.arange(128)
    triu = -(i[:, None] <= i[None, :]).astype(np.float32)      # [s,t] s<=t
    e127 = np.zeros((128, 128), np.float32)
    e127[127, :] = 1.0
    maskneg = np.where(i[:, None] <= i[None, :], 0.0, NEG).astype(np.float32)
    idf = np.eye(128, dtype=np.float32)
    cf32 = np.stack([triu, e127, maskneg, idf], axis=1)
    mask01 = (i[:, None] <= i[None, :]).astype(np.float16)
    cf16 = np.stack([np.eye(128, dtype=np.float16),
                     np.ones((128, 128), np.float16), mask01], axis=1)
    return dict(cf32=np.ascontiguousarray(cf32),
                cf16=np.ascontiguousarray(cf16))


def _bf(x):
    return np.asarray(x, dtype=np.float16)


_NC_CACHE = None


def kernel(x, hidden_state, w_rms, w_qkv, w_gate, w_out):
    global _NC_CACHE
    x = np.asarray(x, np.float32)
    hidden_state = np.asarray(hidden_state, np.float32)
    w_rms = np.asarray(w_rms, np.float32)
    w_qkv = np.asarray(w_qkv, np.float32)
    w_gate = np.asarray(w_gate, np.float32)
    w_out = np.asarray(w_out, np.float32)

    if _NC_CACHE is None:
        _NC_CACHE = build_nc()
    nc = _NC_CACHE

    consts = _consts()
    wq3 = (w_rms[:, None] * w_qkv).reshape(D, 3, H, DH)
    wg3 = (w_rms[:, None] * w_gate).reshape(D, 3, H)

    in_maps = []
    for core in range(8):
        b, hg = core // 2, core % 2
        h0 = hg * HL
        xT = np.ascontiguousarray(x[b].T)                      # [D, S]
        wall = np.concatenate(
            [wq3[:, 0, h0:h0 + HL, :].reshape(D, FL),
             wq3[:, 1, h0:h0 + HL, :].reshape(D, FL),
             wg3[:, :, h0:h0 + HL].reshape(D, 3 * HL)], axis=1)  # [i8|f8|o8]
        m = dict(
            xhi=_bf(xT), wall=_bf(wall),
            wv=_bf(wq3[:, 2, h0:h0 + HL, :].reshape(D, FL)),
            wo=_bf(w_out[h0 * DH:(h0 + HL) * DH, :]),
            s0=_bf(hidden_state[b, h0:h0 + HL]), **consts)
        in_maps.append(m)

    res = bass_utils.run_bass_kernel_spmd(nc, in_maps, core_ids=list(range(8)))

    out = np.empty((B, S, D), np.float32)
    for b in range(B):
        acc = (res.results[2 * b]["outT"].astype(np.float32)
               + res.results[2 * b + 1]["outT"].astype(np.float32))
        out[b] = x[b] + acc.T
    return out



# revision 36
# speedup vs baseline: 1.0086x; 1.0086x over previous
"""mLSTM block kernel for Trainium2, 8 NeuronCores.

Sharding: batch (4) x head-halves (2) -> 8 cores. Each core handles one
batch element and 8 of the 16 heads: RMSNorm + qkv/gate projections +
chunked mLSTM recurrence (chunk=128) + its half of the output projection.
Host adds the two per-batch partial output projections + residual.

Math (per head, chunk of C=128 positions, inclusive cumsums, all
exponent arguments <= 0 so exp never overflows):
  nlogf = softplus(-softcap(gf)),  NL_t = cumsum(nlogf)
  nlogi, nlogo likewise;  Z_t = NL_t + nlogo_t   (folds o_t = exp(-nlogo))
  r_s = NL_s - nlogi_s
  A'[s,t] = (k_s . q_t) * exp(r_s - Z_t)  for t >= s else 0
  h_t = sum_s A'[s,t] v_s + (q_t * exp(-Z_t)) . S_chunk_start
  S <- exp(-NL_C) * S + sum_s exp(NL_s - NL_C - nlogi_s) k_s v_s^T

Decay matrices: dm = min(r_s + (-Z_t), 0) via per-head Pool tensor_scalar
(the min guards exp overflow in the invalid s>t half), Exp on ACT, then a
causal 0/1 mask multiply in fp16 (DVE 4x mode). PSUM evacuations for qkT
and the output projection run on the otherwise-idle Pool engine.

RMSNorm trick: xn = inv[s] * x[:, s], so projections run on raw bf16 x
and are scaled by inv at PSUM-evacuation; w_rms is folded into weights
on the host. Gate projections use bf16 hi/lo splitting (3 matmul
groups) because decay errors random-walk over the 128-step cumsum.
"""

import sys

sys.path.insert(0, "/opt/trn_rl_repo")

import numpy as np
import ml_dtypes

import concourse.bass as bass
import concourse.mybir as mybir
import concourse.tile as tile
from concourse import bass_utils
import bass_rust

F32 = mybir.dt.float32
BF16 = mybir.dt.bfloat16
FP16 = mybir.dt.float16
AF = mybir.ActivationFunctionType
OP = mybir.AluOpType

B, S, D = 4, 2048, 1024
H, DH = 16, 64
HL = 8              # heads per core
C = 128             # chunk length
NCHUNK = S // C     # 16
KO = D // 128       # 8 k-tiles over D
FL = HL * DH        # 512 local feature width
EPS = 1e-6
CAP = 15.0
NEG = -30000.0

# ---------------------------------------------------------------------------
# walrus workaround: this compiler build only accepts 1 sync-wait on
# CTRL-lowered instructions (Drain/EventSemaphore/Nop) and rejects >2
# elsewhere. Move the excess onto EventSemaphore carriers inserted before.
# ---------------------------------------------------------------------------
_CTRL_INSTS = ("InstDrain", "InstEventSemaphore", "InstNop")


def _split_excess_waits(nc, max_waits=1):
    n_new = 0
    for fn in nc.m.functions:
        for bb in fn.blocks:
            lst = bb.instructions
            i = 0
            while i < len(lst):
                inst = lst[i]
                si = inst.sync_info
                cap = 1 if type(inst).__name__ in _CTRL_INSTS else max_waits
                if si is None or len(si.on_wait) <= cap:
                    i += 1
                    continue
                waits = list(si.on_wait)
                keep, excess = waits[:cap], waits[cap:]
                carriers = []
                for j in range(len(excess)):
                    ev = mybir.InstEventSemaphore(
                        name=f"I-waitfix-{n_new}", ins=[], outs=[])
                    n_new += 1
                    ev.engine = inst.engine
                    ev.sync_info = bass_rust.SyncInfo(
                        on_wait=[excess[j]], on_update=[])
                    nc.register_instruction(ev, overwrite=True)
                    carriers.append(ev)
                si.on_wait = keep
                for k, ev in enumerate(carriers):
                    lst.insert(i + k, ev)
                i += len(carriers) + 1
    return n_new


# ---------------------------------------------------------------------------
# kernel builder
# ---------------------------------------------------------------------------

def build_nc():
    nc = bass.Bass(target_bir_lowering=False, trn_type="TRN2")

    # wqkg = [wqk(1024) | wgh(24)] along columns; wv separate (freed early)
    WALL = 2 * FL + 3 * HL
    xhi_d = nc.dram_tensor("xhi", [D, S], FP16, kind="ExternalInput")
    wall_d = nc.dram_tensor("wall", [D, WALL], FP16, kind="ExternalInput")
    wv_d = nc.dram_tensor("wv", [D, FL], FP16, kind="ExternalInput")
    wo_d = nc.dram_tensor("wo", [FL, D], FP16, kind="ExternalInput")
    s0_d = nc.dram_tensor("s0", [HL, DH, DH], FP16, kind="ExternalInput")
    # cf32 = [triu | e127 | maskneg | idf32], cf16 = [idbf | onesb | mask01]
    cf32_d = nc.dram_tensor("cf32", [128, 4, 128], F32, kind="ExternalInput")
    cf16_d = nc.dram_tensor("cf16", [128, 3, 128], FP16, kind="ExternalInput")
    out_d = nc.dram_tensor("outT", [D, S], FP16, kind="ExternalOutput")

    xhi_r = xhi_d.rearrange("(ko p) s -> p ko s", p=128)

    with tile.TileContext(nc) as tc:
        with (
            tc.tile_pool(name="wk", bufs=3) as wk,
            tc.tile_pool(name="dram", bufs=1, space="DRAM") as dpool,
            tc.tile_pool(name="ps_big", bufs=2, space="PSUM") as ps_big,
            tc.tile_pool(name="ps_kt", bufs=2, space="PSUM") as ps_kt,
            tc.tile_pool(name="ps_st", bufs=2, space="PSUM") as ps_st,
        ):
            # ---- persistent tiles --------------------------------------
            frees = []

            def ptile(shape, dtype, name):
                t, f = tc.tile(shape, dtype, name=name)
                frees.append(f)
                return t

            cf32 = ptile([128, 4, 128], F32, "cf32")
            triu, e127, maskneg, idf = (cf32[:, i, :] for i in range(4))
            cf16 = ptile([128, 3, 128], FP16, "cf16")
            idb, onesb, mask01 = (cf16[:, i, :] for i in range(3))
            wall = ptile([128, KO, WALL], FP16, "wall")
            wqk = wall[:, :, 0:2 * FL]
            wgh = wall[:, :, 2 * FL:2 * FL + 3 * HL]
            wo = ptile([128, FL // 128, D], FP16, "wo")
            s_all = ptile([128, 2, HL // 2, DH], FP16, "s_all")
            li1_cols = ptile([128, NCHUNK], F32, "li1_cols")
            li2_cols = ptile([128, NCHUNK], F32, "li2_cols")
            gsp = ptile([128, NCHUNK, 24], F32, "gsp")
            nl_cols = ptile([128, 128], F32, "nl_cols")
            r_cols = ptile([128, 128], F32, "r_cols")
            z_cols = ptile([128, 128], F32, "z_cols")
            w_cols = ptile([128, 128], F32, "w_cols")
            exp_nlc = ptile([128, 128], F32, "exp_nlc")
            qkT = ptile([128, KO, S], FP16, "qkT")
            vpos = ptile([128, NCHUNK, FL], FP16, "vpos")
            eps_col = ptile([128, 1], F32, "eps_col")
            nc.vector.memset(eps_col[:], EPS)
            xhi, xhi_free = tc.tile([128, KO, S], FP16, name="xhi")
            wv, wv_free = tc.tile([128, KO, FL], FP16, name="wv")
            # freed before hT allocation (only needed for decay tables)
            lnv_rep, lnv_free = tc.tile([128, S], F32, name="lnv_rep")
            g_cols, g_free = tc.tile([128, NCHUNK, 24], F32, name="g_cols")

            # ---- loads (ordered so PE can start earliest) --------------
            nc.sync.dma_start(cf16[:], cf16_d[:])
            # first s-block of x unblocks variance-sb0 at ~3us
            nc.sync.dma_start(xhi[:, :, 0:512], xhi_r[:, :, 0:512])
            nc.sync.dma_start(cf32[:], cf32_d[:])
            wall_r = wall_d.rearrange("(ko p) j -> p ko j", p=128)
            nc.sync.dma_start(wall[:, :, 0:512], wall_r[:, :, 0:512])
            nc.sync.dma_start(wall[:, :, 512:WALL], wall_r[:, :, 512:WALL])
            for sb in range(1, 4):
                nc.sync.dma_start(xhi[:, :, sb * 512:(sb + 1) * 512],
                                  xhi_r[:, :, sb * 512:(sb + 1) * 512])
            nc.sync.dma_start(wv[:], wv_d.rearrange("(ko p) j -> p ko j", p=128))
            nc.sync.dma_start(wo[:], wo_d.rearrange("(ko p) j -> p ko j", p=128))
            # state init: head h -> partitions (h%2)*64+d, index h//2
            nc.sync.dma_start(
                s_all[:, 0, :, :],
                s0_d.rearrange("(a b) d e -> (b d) a e", b=2))

            # ---- projection group emitters -----------------------------
            def emit_qk_group(jt, sb):
                pq = ps_big.tile([128, 512], F32, name="pq", tag="big")
                for k in range(KO):
                    nc.tensor.matmul(
                        pq[:], wqk[:, k, jt * 128:(jt + 1) * 128],
                        xhi[:, k, sb * 512:(sb + 1) * 512],
                        start=(k == 0), stop=(k == KO - 1))
                nc.scalar.copy(
                    qkT[:, jt, sb * 512:(sb + 1) * 512], pq[:])

            def emit_vpos_group(st):
                pvp = ps_big.tile([128, 512], F32, name="pvp", tag="big")
                for k in range(KO):
                    nc.tensor.matmul(
                        pvp[:], xhi[:, k, st * 128:(st + 1) * 128],
                        wv[:, k, :],
                        start=(k == 0), stop=(k == KO - 1))
                nc.scalar.copy(vpos[:, st, :], pvp[:])

            # ---- variance + inv std (sb0 first: only needs xhi-sb0) ----
            def emit_var(sb):
                pv = ps_kt.tile([128, 512], F32, name="pv", tag="kt")
                for k in range(KO):
                    xsq = wk.tile([128, 512], FP16, name="xsq", tag="xsq", bufs=2)
                    xs = xhi[:, k, sb * 512:(sb + 1) * 512]
                    nc.vector.tensor_tensor(xsq[:], xs, xs, OP.mult)
                    nc.tensor.matmul(pv[:], onesb, xsq[:],
                                     start=(k == 0), stop=(k == KO - 1))
                nc.scalar.activation(lnv_rep[:, sb * 512:(sb + 1) * 512],
                                     pv[:], AF.Ln, bias=eps_col[:],
                                     scale=1.0 / D)

            emit_var(0)

            # q|k projection for the first s-block, ahead of everything
            for jt in range(KO):
                emit_qk_group(jt, 0)

            for sb in range(1, 4):
                emit_var(sb)

            # ln(var)/D columns for decay-table folding: [128 s, st]
            for st in range(NCHUNK):
                pt = ps_kt.tile([128, 128], F32, name="pt", tag="kt")
                nc.tensor.transpose(
                    pt[:, 0:1], lnv_rep[0:1, st * 128:(st + 1) * 128],
                    cf32[0:1, 3, 0:1])
                # ln(inv) = -0.5*lnv ; store -2*ln(inv)=lnv and -ln(inv)
                nc.scalar.activation(li1_cols[:, st:st + 1], pt[:, 0:1],
                                     AF.Copy, scale=-1.0)
                nc.scalar.activation(li2_cols[:, st:st + 1], pt[:, 0:1],
                                     AF.Copy, scale=-0.5)

            # ---- gate projections (row-major, hi/lo compensated) -------
            for sb in range(4):
                pg = ps_kt.tile([24, 512], F32, name="pg", tag="kt")
                for k in range(KO):
                    nc.tensor.matmul(pg[:], wgh[:, k, :],
                                     xhi[:, k, sb * 512:(sb + 1) * 512],
                                     start=(k == 0), stop=(k == KO - 1))
                inv24 = wk.tile([24, 512], F32, name="inv24", tag="inv24",
                                bufs=2)
                nc.scalar.activation(inv24[:],
                                     lnv_rep[0:24, sb * 512:(sb + 1) * 512],
                                     AF.Exp, scale=-0.5)
                graw = wk.tile([24, 512], F32, name="graw", tag="graw",
                               bufs=2)
                nc.vector.tensor_tensor(graw[:], pg[:], inv24[:], OP.mult)
                for j in range(4):
                    st = sb * 4 + j
                    pt = ps_kt.tile([128, 128], F32, name="pt", tag="kt")
                    nc.tensor.transpose(
                        pt[:, 0:24], graw[:, j * 128:(j + 1) * 128],
                        cf32[0:24, 3, 0:24])
                    nc.scalar.copy(g_cols[:, st, :], pt[:, 0:24])
            # softcap -> log-gates: gsp = ln(sigmoid(15*tanh(g/15))) <= 0
            # (CoreSim lacks Softplus; -gsp is the neg-log gate)
            t1 = wk.tile([128, NCHUNK, 24], F32, name="t1", tag="t1",
                         bufs=1)
            nc.scalar.activation(t1[:], g_cols[:], AF.Tanh, scale=1.0 / CAP)
            sg = wk.tile([128, NCHUNK, 24], F32, name="sg", tag="sg",
                         bufs=1)
            nc.scalar.activation(sg[:], t1[:], AF.Sigmoid, scale=CAP)
            nc.scalar.activation(gsp[:], sg[:], AF.Ln)
            lns_i = gsp[:, :, 0:HL]
            lns_f = gsp[:, :, HL:2 * HL]
            lns_o = gsp[:, :, 2 * HL:3 * HL]

            # ---- cumsums / decay tables --------------------------------
            # triu holds -1 on s<=t, so NL = -cumsum(ln f) >= 0
            pnl = ps_kt.tile([128, 512], F32, name="pnl", tag="kt")
            nc.tensor.matmul(pnl[:, 0:128], triu, lns_f,
                             start=True, stop=True)
            nc.vector.tensor_copy(nl_cols[:], pnl[:, 0:128])
            nc.vector.tensor_tensor(r_cols[:], nl_cols[:], lns_i, OP.add)
            nc.vector.tensor_tensor(
                r_cols[:], r_cols[:],
                li1_cols[:, :, None].to_broadcast((128, NCHUNK, HL)),
                OP.add)
            # z_cols holds the NEGATED exponent -Z = lns_o - NL + ln(inv)
            nc.vector.tensor_tensor(z_cols[:], lns_o, nl_cols[:], OP.subtract)
            nc.vector.tensor_tensor(
                z_cols[:], z_cols[:],
                li2_cols[:, :, None].to_broadcast((128, NCHUNK, HL)),
                OP.add)
            pnlc = ps_kt.tile([128, 512], F32, name="pnlc", tag="kt")
            nc.tensor.matmul(pnlc[:, 0:128], e127, nl_cols[:],
                             start=True, stop=True)
            w_tmp = wk.tile([128, 128], F32, name="w_tmp", tag="tmp")
            nc.vector.tensor_tensor(w_tmp[:], r_cols[:], pnlc[:, 0:128],
                                    OP.subtract)
            nc.scalar.activation(w_cols[:], w_tmp[:], AF.Exp)
            nc.scalar.activation(exp_nlc[:], pnlc[:, 0:128], AF.Exp,
                                 scale=-1.0)
            # -Z rows -> DRAM for partition-broadcast loads
            pzr = ps_kt.tile([128, 128], F32, name="pzr", tag="kt")
            nc.tensor.transpose(pzr[:], z_cols[:], idf)
            zr_sb = wk.tile([128, 128], F32, name="zr_sb", tag="tmp")
            nc.vector.tensor_copy(zr_sb[:], pzr[:])
            zr_dram = dpool.tile([128, 128], F32, name="zr_dram")
            nc.sync.dma_start(zr_dram[:], zr_sb[:])

            # ---- v projection: two chunks ahead; rest dripped in-loop --
            for st in range(2):
                emit_vpos_group(st)

            g_free()
            lnv_free()
            hT, hT_free = tc.tile([128, FL // 128, S], FP16, name="hT")

            # ---- recurrence over chunks --------------------------------
            # waves of 4 heads sharing a partition window: h = 2k + h0
            w_v = w_cols.rearrange("s (c k two) -> s c k two", k=4, two=2)
            e_v = exp_nlc.rearrange("s (c k two) -> s c k two", k=4, two=2)
            for c in range(NCHUNK):
                if c % 2 == 0:
                    repz = wk.tile([128, 2, 4, 2, 128], F32, name="repz",
                                   tag="repz", bufs=2)
                    nc.sync.dma_start(
                        repz[:],
                        zr_dram[None, c * HL:(c + 2) * HL, :]
                        .to_broadcast((128, 2 * HL, 128))
                        .rearrange("s (a k two) t -> s a k two t", k=4, two=2))
                    repez = wk.tile([128, 2, 4, 2, 128], FP16, name="repez",
                                    tag="repez", bufs=2)
                    nc.scalar.activation(repez[:], repz[:], AF.Exp)
                cs = slice(c * 128, (c + 1) * 128)
                # decay matrices: dm = min(r_s + (-Z_t), 0) per head on Pool
                # (min guards exp overflow on the invalid s>t half), then
                # exp on ACT and causal 0/1 mask in fp16 (DVE 4x).
                dm = wk.tile([128, 4, 2, 128], F32, name="dm", tag="d1",
                             bufs=2)
                for k in range(4):
                    for two in range(2):
                        col = c * 8 + k * 2 + two
                        nc.gpsimd.tensor_scalar(
                            dm[:, k, two, :], repz[:, c % 2, k, two, :],
                            r_cols[:, col:col + 1], 0.0, OP.add, OP.min)
                expd = wk.tile([128, 4, 2, 128], FP16, name="expd",
                               tag="expd", bufs=3)
                nc.scalar.activation(expd[:], dm[:], AF.Exp)
                nc.vector.tensor_tensor(
                    expd[:], expd[:],
                    mask01[:, None, None, :].to_broadcast((128, 4, 2, 128)),
                    OP.mult)
                for h0 in range(2):
                    po = h0 * 64
                    pw = slice(po, po + 64)
                    qs_w = qkT[pw, 0:4, cs]
                    ks_w = qkT[pw, 4:8, cs]
                    rez_w = repez[pw, c % 2, :, h0, :]
                    wc_bc = w_v[:, c, :, h0, None].to_broadcast((128, 4, 64))
                    en_bc = e_v[pw, c, :, h0, None].to_broadcast((64, 4, 64))
                    s_old_w = s_all[pw, c % 2, :, :]
                    s_new_w = s_all[pw, (c + 1) % 2, :, :]

                    # attention scores (k.q), 4 heads into one bank
                    pa = ps_big.tile([128, 4, 128], F32, name="pa", tag="pa")
                    for k in range(4):
                        nc.tensor.matmul(pa[:, k, :], ks_w[:, k, :],
                                         qs_w[:, k, :], start=True, stop=True,
                                         skip_group_check=True)
                    pkp = ps_st.tile([128, 4, DH], FP16, name="pkp",
                                     tag="st2")
                    for k in range(4):
                        nc.tensor.matmul(pkp[:, k, :], ks_w[:, k, :],
                                         idb[pw, pw], is_transpose=True,
                                         skip_group_check=True)
                    a_w = wk.tile([128, 4, 128], FP16, name="a_w", tag="a_w",
                                  bufs=2)
                    nc.vector.tensor_tensor(a_w[:], pa[:],
                                            expd[:, :, h0, :], OP.mult)
                    qt_w = wk.tile([128, 4, 128], FP16, name="qt_w",
                                   tag="qt_w", bufs=2)
                    nc.vector.tensor_tensor(qt_w[pw, :, :], qs_w, rez_w,
                                            OP.mult)

                    # hT[e,t] = V^T A' + S^T (q*exp(-Z))
                    pht = ps_kt.tile([128, 4, 128], F32, name="pht",
                                     tag="kt")
                    for k in range(4):
                        vs = vpos[:, c, (2 * k + h0) * DH:
                                  (2 * k + h0 + 1) * DH]
                        nc.tensor.matmul(pht[pw, k, :], vs, a_w[:, k, :],
                                         start=True, stop=False,
                                         tile_position=(0, po),
                                         skip_group_check=True)
                        nc.tensor.matmul(pht[pw, k, :], s_old_w[:, k, :],
                                         qt_w[pw, k, :],
                                         start=False, stop=True,
                                         tile_position=(po, po),
                                         skip_group_check=True)
                    nc.scalar.copy(hT[pw, :, cs], pht[pw, :, :])

                    # state update: S <- exp(-NL_C) S + K~^T V
                    ktb = wk.tile([128, 4, DH], FP16, name="ktb", tag="ktb")
                    nc.vector.tensor_tensor(ktb[:], pkp[:], wc_bc, OP.mult)
                    pst = ps_st.tile([128, 4, DH], F32, name="pst",
                                     tag="st2")
                    for k in range(4):
                        vs = vpos[:, c, (2 * k + h0) * DH:
                                  (2 * k + h0 + 1) * DH]
                        nc.tensor.matmul(pst[pw, k, :], ktb[:, k, :], vs,
                                         start=True, stop=True,
                                         tile_position=(0, po),
                                         skip_group_check=True)
                    sdec = wk.tile([128, 4, DH], F32, name="sdec", tag="sdec")
                    nc.gpsimd.tensor_tensor(sdec[pw, :, :], s_old_w, en_bc,
                                            OP.mult)
                    nc.vector.tensor_tensor(s_new_w, sdec[pw, :, :],
                                            pst[pw, :, :], OP.add)

                # drip-feed v projection two chunks ahead
                if c < NCHUNK - 2:
                    emit_vpos_group(c + 2)

                # drip-feed remaining q|k projection groups (sb = 1..4)
                if c < 12:
                    sb_n = 1 + c // 4
                    emit_qk_group(2 * (c % 4), sb_n)
                    emit_qk_group(2 * (c % 4) + 1, sb_n)

                # interleave output projection once its s-range is complete.
                # sb3 is split in half (c=13 and c=15) to shrink the tail.
                def emit_out(lo, width):
                    osb = wk.tile([128, KO, width], FP16, name="osb",
                                  tag="osb", bufs=1)
                    for jt in range(KO):
                        pout = ps_big.tile([128, width], F32, name="pout",
                                           tag="big")
                        for kk in range(FL // 128):
                            nc.tensor.matmul(
                                pout[:], wo[:, kk, jt * 128:(jt + 1) * 128],
                                hT[:, kk, lo:lo + width],
                                start=(kk == 0), stop=(kk == FL // 128 - 1))
                        nc.vector.tensor_copy(osb[:, jt, :], pout[:])
                    nc.sync.dma_start(
                        out_d.rearrange("(jt p) s -> p jt s", p=128)
                        [:, :, lo:lo + width], osb[:])

                if c in (3, 7, 11):
                    emit_out((c // 4) * 512, 512)
                elif c == 13:
                    emit_out(1536, 256)
                elif c == 15:
                    emit_out(1792, 256)

            hT_free()
            wv_free()
            xhi_free()
            for f in reversed(frees):
                f()

    _split_excess_waits(nc)
    nc.finalize()
    return nc


# ---------------------------------------------------------------------------
# host-side constants and shard prep
# ---------------------------------------------------------------------------

def _consts():
    i = np.arange(128)
    triu = -(i[:, None] <= i[None, :]).astype(np.float32)      # [s,t] s<=t
    e127 = np.zeros((128, 128), np.float32)
    e127[127, :] = 1.0
    maskneg = np.where(i[:, None] <= i[None, :], 0.0, NEG).astype(np.float32)
    idf = np.eye(128, dtype=np.float32)
    cf32 = np.stack([triu, e127, maskneg, idf], axis=1)
    mask01 = (i[:, None] <= i[None, :]).astype(np.float16)
    cf16 = np.stack([np.eye(128, dtype=np.float16),
                     np.ones((128, 128), np.float16), mask01], axis=1)
    return dict(cf32=np.ascontiguousarray(cf32),
                cf16=np.ascontiguousarray(cf16))


def _bf(x):
    return np.asarray(x, dtype=np.float16)


_NC_CACHE = None


def kernel(x, hidden_state, w_rms, w_qkv, w_gate, w_out):
    global _NC_CACHE
    x = np.asarray(x, np.float32)
    hidden_state = np.asarray(hidden_state, np.float32)
    w_rms = np.asarray(w_rms, np.float32)
    w_qkv = np.asarray(w_qkv, np.float32)
    w_gate = np.asarray(w_gate, np.float32)
    w_out = np.asarray(w_out, np.float32)

    if _NC_CACHE is None:
        _NC_CACHE = build_nc()
    nc = _NC_CACHE

    consts = _consts()
    wq3 = (w_rms[:, None] * w_qkv).reshape(D, 3, H, DH)
    wg3 = (w_rms[:, None] * w_gate).reshape(D, 3, H)

    in_maps = []
    for core in range(8):
        b, hg = core // 2, core % 2
        h0 = hg * HL
        xT = np.ascontiguousarray(x[b].T)                      # [D, S]
        wall = np.concatenate(
            [wq3[:, 0, h0:h0 + HL, :].reshape(D, FL),
             wq3[:, 1, h0:h0 + HL, :].reshape(D, FL),
             wg3[:, :, h0:h0 + HL].reshape(D, 3 * HL)], axis=1)  # [i8|f8|o8]
        m = dict(
            xhi=_bf(xT), wall=_bf(wall),
            wv=_bf(wq3[:, 2, h0:h0 + HL, :].reshape(D, FL)),
            wo=_bf(w_out[h0 * DH:(h0 + HL) * DH, :]),
            s0=_bf(hidden_state[b, h0:h0 + HL]), **consts)
        in_maps.append(m)

    res = bass_utils.run_bass_kernel_spmd(nc, in_maps, core_ids=list(range(8)))

    out = np.empty((B, S, D), np.float32)
    for b in range(B):
        acc = (res.results[2 * b]["outT"].astype(np.float32)
               + res.results[2 * b + 1]["outT"].astype(np.float32))
        out[b] = x[b] + acc.T
    return out

